# revision 29
# baseline (speedup 1.0000x reference)
"""Trainium2 Bass kernel for nn_GAT_1675037246077 (2-layer GAT + linear head).

Strategy (8 NeuronCores, SPMD single NEFF):
 - Destination-sharded: core c owns dst nodes [c*12544, (c+1)*12544); nodes padded
   to NPAD = 100352 = 8*128*98 (pad nodes x=0, degree 0). Per-core nodes sorted by
   max-per-chunk in-degree (desc) and packed into 98 tiles of 128 nodes.
 - Edge grids EXCLUDE self-loops (self terms use resident own-node data).
 - Layer 1: host-expanded x in edge-slot order -> per-slot [16,128]@[16,99]
   matmuls (5 slots share a PSUM tile + one copy). Grid [128, J1_t] per tile.
 - Layer 2: table2[NPAD, 128] = [h2(32)|as2(1)|0...] built shard-wise,
   AllGather'd, then gathered per edge via InstDMAGatherAnt (dma_gather) on 4
   SWDGE queues: 256B rows, int16 indices local to 4 source-position chunks of
   25088 rows. Grid per tile: 4 chunk-pure segments of J4u_t columns each
   (chunk-major within group); <=1024 indices per call (descriptor carveout).
 - Scores exp(leakyrelu(s)) = exp(0.2*(4*relu(s)+s)); weighted aggregation via
   identity-matmul PSUM accumulation; exact pad-slot corrections (all pad rows
   have identical table2 content); softmax divide; bias+selu chunked epilogue.
"""
import numpy as np
import ml_dtypes

from concourse import mybir, tile, bacc
import concourse.bass as bass
from concourse.bass_utils import run_bass_kernel_spmd
from concourse.masks import make_identity

P = 128
AF = mybir.ActivationFunctionType
ALU = mybir.AluOpType
BF16 = mybir.dt.bfloat16
F32 = mybir.dt.float32
I16 = mybir.dt.int16
NPBF16 = ml_dtypes.bfloat16

SELU_SCALE = 1.0507009873554805
SELU_ALPHA_SCALE = 1.7580993408473766

NCHUNK = 4
TB2 = 128          # padded table2 row (bf16 cols) = 256B
MAXC = 8           # max columns per dma_gather call (1024 idxs)


class Cfg:
    def __init__(self, N, E, ncores, fin=16, h1=3, c1=32, c2=32, ncout=16):
        self.N, self.E, self.ncores = N, E, ncores
        self.FIN, self.H1, self.C1, self.C2, self.NCOUT = fin, h1, c1, c2, ncout
        self.CW1 = h1 * c1              # 96
        self.F1 = self.CW1 + h1         # table1 cols (99)
        self.SH = ((N + ncores - 1) // ncores + P - 1) // P * P
        self.TPC = self.SH // P
        self.NPAD = self.SH * ncores
        self.CH = self.NPAD // NCHUNK   # 25088
        self.WAUG2_C = c2 + 2           # [W2(32)|as2|ad2]


def _group_plan(J, cap, max_nt):
    groups = []
    t = 0
    n = len(J)
    while t < n:
        j = J[t]
        nt = 1
        while t + nt < n and J[t + nt] == j and (nt + 1) * j <= cap and nt < max_nt:
            nt += 1
        groups.append((t, nt, int(j)))
        t += nt
    return groups


def preprocess(cfg, x, edge_index, W1, a_src1, a_dst1, W2, a_src2, a_dst2):
    N, E, NC = cfg.N, cfg.E, cfg.ncores
    SH, TPC, NPAD, CH = cfg.SH, cfg.TPC, cfg.NPAD, cfg.CH

    src = edge_index[0].astype(np.int64)
    dst = edge_index[1].astype(np.int64)
    deg = np.bincount(dst, minlength=NPAD)

    # interleaved sharding: node n -> core n % NC (spreads pad nodes to all
    # cores so every chunk's last position is a zero pad row).
    src_chunk = (src % NC) // 2
    cnt = np.zeros((NPAD, NCHUNK), np.int64)
    np.add.at(cnt, (dst, src_chunk), 1)
    mx = cnt.max(1)
    key = -(mx * 1000000 + deg)

    perms = []
    pos = np.empty(NPAD, np.int64)
    J1_all = np.zeros((NC, TPC), np.int64)
    J4_all = np.zeros((NC, TPC), np.int64)
    for c in range(NC):
        ids = np.arange(c, NPAD, NC)
        order = np.argsort(key[ids], kind="stable")
        perm = ids[order]
        perms.append(perm)
        pos[perm] = c * SH + np.arange(SH)
        pc = cnt[perm].reshape(TPC, P, NCHUNK)
        J1_all[c] = pc.sum(axis=2).max(axis=1)
        J4_all[c] = pc.max(axis=1).max(axis=1)
    J1 = np.maximum(J1_all.max(0), 1)
    J4u = np.maximum(J4_all.max(0), 1)
    SLOTS = int(J1.sum())
    NC4 = int(J4u.sum())        # columns per chunk in the L2 grid
    offs1 = np.zeros(TPC + 1, np.int64)
    np.cumsum(J1, out=offs1[1:])
    offs4 = np.zeros(TPC + 1, np.int64)
    np.cumsum(J4u, out=offs4[1:])

    e_order = np.argsort(dst, kind="stable")
    src_sorted = src[e_order]
    starts = np.zeros(NPAD + 1, np.int64)
    np.cumsum(deg, out=starts[1:])

    FIN, H1, C1, CW1 = cfg.FIN, cfg.H1, cfg.C1, cfg.CW1
    waug1 = np.zeros((FIN, CW1 + 2 * H1), np.float32)
    waug1[:, :CW1] = W1
    for h in range(H1):
        waug1[:, CW1 + h] = W1[:, h * C1:(h + 1) * C1] @ a_src1[h]
        waug1[:, CW1 + H1 + h] = W1[:, h * C1:(h + 1) * C1] @ a_dst1[h]
    C2 = cfg.C2
    waug2 = np.zeros((CW1, cfg.WAUG2_C), np.float32)
    waug2[:, :C2] = W2
    waug2[:, C2] = W2[:, :] @ a_src2[0]
    waug2[:, C2 + 1] = W2[:, :] @ a_dst2[0]

    xpad = np.zeros((NPAD, FIN), np.float32)
    xpad[:N] = x
    xw = xpad @ waug1
    xpadT_bf = np.ascontiguousarray(xpad.T).astype(NPBF16)

    # L2 group + call plan (shared across cores; SPMD constants)
    groups2 = _group_plan(list(J4u), 16, 8)
    calls2 = []   # per group: list of (k, c0, nc, blk)
    nblk = 0
    for (t0, nt, Jg) in groups2:
        cl = []
        cols = nt * Jg
        for k in range(NCHUNK):
            c0 = 0
            while c0 < cols:
                ncol = min(MAXC, cols - c0)
                cl.append((k, c0, ncol, nblk))
                nblk += 1
                c0 += ncol
        calls2.append(cl)

    def pslice(vals, w):
        return np.ascontiguousarray(
            vals.reshape(TPC, P, w).transpose(1, 0, 2).reshape(P, TPC * w))

    def bf(v):
        return v.astype(NPBF16).astype(np.float32)

    percore = []
    for c in range(NC):
        perm = perms[c]
        idx1 = np.full((P, SLOTS), NPAD - 1, np.int32)
        npad2 = np.zeros((P, TPC), np.float32)
        idxL2 = np.full((NCHUNK, P, NC4), CH - 1, np.int64)
        for t in range(TPC):
            jt = int(J1[t])
            o1 = int(offs1[t])
            o4 = int(offs4[t])
            for p in range(P):
                node = perm[t * P + p]
                dg = int(deg[node])
                s0 = int(starts[node])
                ss = src_sorted[s0:s0 + dg]
                idx1[p, o1:o1 + dg] = ss
                sp = pos[ss]
                ck = sp // CH
                for k in range(NCHUNK):
                    sel = sp[ck == k] - k * CH
                    idxL2[k, p, o4:o4 + len(sel)] = sel
            npad2[:, t] = NCHUNK * int(J4u[t])
        # subtract real degree to get pad counts
        dgp = deg[perm].reshape(TPC, P).T  # [P, TPC]
        npad2 -= dgp
        # wrapped int16 idx blocks, in call order
        idxg = np.zeros((16, nblk * 64), np.int16)
        for gi, (t0, nt, Jg) in enumerate(groups2):
            for (k, c0, ncol, blk) in calls2[gi]:
                g0 = int(offs4[t0]) + c0
                seq = idxL2[k][:, g0:g0 + ncol].T.ravel()  # p fastest
                w = seq.reshape(-1, 16).T.astype(np.int16)  # [16, ncol*8]
                idxg[:, blk * 64: blk * 64 + ncol * 8] = w
        idxg_full = np.tile(idxg, (8, 1))

        cols1 = idx1.T.ravel()
        xexpT = np.ascontiguousarray(xpadT_bf[:, cols1])
        ad1v = xw[perm, CW1 + H1:CW1 + 2 * H1]
        as1v = xw[perm, CW1:CW1 + H1]
        sv = (as1v + ad1v).astype(np.float32)
        p1 = bf(np.maximum(np.exp(sv), np.exp(0.2 * sv)))
        a1f = ad1v.astype(np.float32)
        c1 = bf(np.maximum(np.exp(a1f), np.exp(0.2 * a1f)))
        npad1 = np.zeros((P, TPC), np.float32)
        npad1[:] = J1[None, :].astype(np.float32)
        npad1 -= dgp
        zc1 = (npad1[:, :, None] * pslice(c1, H1).reshape(P, TPC, H1)
               - pslice(p1, H1).reshape(P, TPC, H1)).reshape(P, TPC * H1)
        h1v = xw[perm, :CW1]
        selfv1 = pslice((p1[:, :, None] * h1v.reshape(SH, H1, C1)).reshape(SH, CW1),
                        CW1)
        percore.append(dict(
            npad2=npad2, own=perm.astype(np.int64),
            xexpT=xexpT, idxg=idxg_full,
            ad1=pslice(ad1v, H1).astype(np.float32),
            zc1=np.ascontiguousarray(zc1, np.float32),
            selfv1=selfv1.astype(NPBF16)))

    meta = dict(J1=[int(j) for j in J1], offs1=[int(o) for o in offs1],
                J4u=[int(j) for j in J4u], offs4=[int(o) for o in offs4],
                SLOTS=SLOTS, NC4=NC4, groups2=groups2, calls2=calls2, nblk=nblk)
    return percore, waug1, waug2, meta


def build_nc(cfg, meta, debug=False):
    J1, offs1 = meta["J1"], meta["offs1"]
    SLOTS, nblk = meta["SLOTS"], meta["nblk"]
    groups2, calls2 = meta["groups2"], meta["calls2"]
    TPC, NPAD, SH, CH = cfg.TPC, cfg.NPAD, cfg.SH, cfg.CH
    FIN, H1, CW1, C2, F1 = cfg.FIN, cfg.H1, cfg.CW1, cfg.C2, cfg.F1
    NCOUT = cfg.NCOUT
    groups1 = _group_plan(J1, 48, 6)

    nc = bacc.Bacc("TRN2", target_bir_lowering=False, debug=debug,
                   num_devices=cfg.ncores, num_swdge_queues=4)

    t_xexpT = nc.dram_tensor("xexpT", [FIN, SLOTS * P], BF16, kind="ExternalInput")
    t_selfv1 = nc.dram_tensor("selfv1", [P, TPC * CW1], BF16, kind="ExternalInput")
    t_idxg = nc.dram_tensor("idxg", [P, nblk * 64], I16, kind="ExternalInput")
    t_waug1 = nc.dram_tensor("waug1", [FIN, CW1 + 2 * H1], BF16, kind="ExternalInput")
    t_waug2 = nc.dram_tensor("waug2", [CW1, cfg.WAUG2_C], BF16, kind="ExternalInput")
    t_wf = nc.dram_tensor("wf", [C2, NCOUT], BF16, kind="ExternalInput")
    t_npad2 = nc.dram_tensor("npad2", [P, TPC], F32, kind="ExternalInput")
    t_ad1 = nc.dram_tensor("ad1", [P, TPC * H1], F32, kind="ExternalInput")
    t_zc1 = nc.dram_tensor("zc1", [P, TPC * H1], F32, kind="ExternalInput")
    t_b1r = nc.dram_tensor("b1r", [P, CW1], F32, kind="ExternalInput")
    t_b2r = nc.dram_tensor("b2r", [P, C2], F32, kind="ExternalInput")
    t_bfr = nc.dram_tensor("bfr", [P, NCOUT], F32, kind="ExternalInput")
    t_out = nc.dram_tensor("out", [SH, NCOUT], F32, kind="ExternalOutput")

    t_cc_in = nc.dram_tensor("cc_in", [SH, TB2], BF16)
    cc_space = "Shared" if cfg.ncores > 4 else "Local"
    t_cc_out = nc.dram_tensor("cc_out", [NPAD, TB2], BF16, addr_space=cc_space)

    with tile.TileContext(nc) as tc:
        with (
            tc.tile_pool(name="res", bufs=1) as res,
            tc.tile_pool(name="pa", bufs=3) as pa,
            tc.tile_pool(name="pb", bufs=2) as pb,
            tc.tile_pool(name="pg", bufs=3) as pg,
            tc.tile_pool(name="fin", bufs=2) as fin,
            tc.tile_pool(name="ep", bufs=1) as ep,
            tc.tile_pool(name="psA", bufs=2, space="PSUM") as psA,
            tc.tile_pool(name="acc", bufs=4, space="PSUM") as accp,
            tc.tile_pool(name="aux", bufs=2, space="PSUM") as auxp,
        ):
            ident = res.tile([P, P], BF16)
            make_identity(nc, ident[:])
            waug1 = res.tile([FIN, CW1 + 2 * H1], BF16)
            nc.sync.dma_start(waug1[:], t_waug1[:, :])
            waug2 = res.tile([CW1, cfg.WAUG2_C], BF16)
            nc.sync.dma_start(waug2[:], t_waug2[:, :])
            wf = res.tile([C2, NCOUT], BF16)
            nc.sync.dma_start(wf[:], t_wf[:, :])
            npad2 = res.tile([P, TPC], F32)
            nc.sync.dma_start(npad2[:], t_npad2[:, :])
            b1r = res.tile([P, CW1], F32)
            nc.sync.dma_start(b1r[:], t_b1r[:, :])
            b2r = res.tile([P, C2], F32)
            nc.sync.dma_start(b2r[:], t_b2r[:, :])
            bfr = res.tile([P, NCOUT], F32)
            nc.sync.dma_start(bfr[:], t_bfr[:, :])
            ad1 = res.tile([P, TPC * H1], F32)
            nc.sync.dma_start(ad1[:], t_ad1[:, :])
            zc1 = res.tile([P, TPC * H1], F32)
            nc.sync.dma_start(zc1[:], t_zc1[:, :])
            ad2 = res.tile([P, TPC], F32)
            hoa = res.tile([P, TPC * (C2 + 1)], BF16)  # [h2own(32)|as2own]
            zc2 = res.tile([P, TPC], F32)
            nc2 = res.tile([P, TPC], F32)
            o1 = res.tile([P, TPC * CW1], BF16)   # pre-selu; reused as h2in
            o2 = res.tile([P, TPC * C2], BF16)    # pre-selu; reused as h3

            # ---- layer 1 (mm expansion) ----
            _layer1(nc, pb, fin, accp, psA, groups1, offs1, t_xexpT, waug1,
                    F1, CW1, H1, ad1, zc1, ident, o1, t_selfv1)
            _selu_epilogue(nc, ep, o1, b1r, o1, TPC, CW1)
            h2in = o1

            # ---- table2 build + AllGather ----
            def build_t2(t):
                tp = auxp.tile([CW1, P], BF16, tag="ps_tp")
                nc.tensor.transpose(tp[:], h2in[:, t * CW1:(t + 1) * CW1], ident[:])
                h2T = pa.tile([CW1, P], BF16, tag="h2T")
                nc.scalar.activation(h2T[:], tp[:], AF.Copy)
                ps2 = psA.tile([P, 512], F32, tag="ps_small")
                nc.tensor.matmul(ps2[:, :cfg.WAUG2_C], lhsT=h2T[:], rhs=waug2[:],
                                 start=True, stop=True)
                nc.vector.tensor_copy(ad2[:, t:t + 1], ps2[:, C2 + 1:C2 + 2])
                nc.scalar.activation(hoa[:, t * (C2 + 1):(t + 1) * (C2 + 1)],
                                     ps2[:, :C2 + 1], AF.Copy)
                st2 = pa.tile([P, C2 + 1], BF16, tag="st2")
                nc.scalar.activation(st2[:], ps2[:, :C2 + 1], AF.Copy)
                nc.sync.dma_start(t_cc_in[t * P:(t + 1) * P, 0:C2 + 1], st2[:])

            for t in range(TPC):
                build_t2(t)

            s2p = pb.tile([P, TPC], F32, tag="s2c")
            hoav = hoa[:].rearrange("p (t k) -> p t k", k=C2 + 1)
            nc.vector.tensor_tensor(out=s2p[:][:, :, None], in0=hoav[:, :, C2:C2 + 1],
                                    in1=ad2[:][:, :, None], op=ALU.add)
            p2 = res.tile([P, TPC], BF16)
            _lrelu_exp(nc, pb, p2[:], s2p[:], [P, TPC])
            selfv2 = res.tile([P, TPC * C2], BF16)
            nc.vector.tensor_tensor(
                out=selfv2[:].rearrange("p (t c) -> p t c", c=C2),
                in0=hoav[:, :, 0:C2],
                in1=p2[:][:, :, None].to_broadcast([P, TPC, C2]),
                op=ALU.mult)
            nc.gpsimd.collective_compute(
                "AllGather", ALU.bypass,
                replica_groups=[list(range(cfg.ncores))],
                ins=[t_cc_in.ap().opt()],
                outs=[t_cc_out.ap().opt()],
            )

            rep2 = res.tile([P, C2 + 1], F32)
            rep_src = bass.AP(tensor=t_cc_out.ap().tensor, offset=(NPAD - 1) * TB2,
                              ap=[[0, P], [1, C2 + 1]])
            nc.gpsimd.dma_start(out=rep2[:], in_=rep_src)

            s2 = pb.tile([P, TPC], F32, tag="s2c")
            nc.vector.tensor_tensor(out=s2[:], in0=ad2[:],
                                    in1=rep2[:, C2:C2 + 1].to_broadcast([P, TPC]),
                                    op=ALU.add)
            c2b = res.tile([P, TPC], BF16)
            _lrelu_exp(nc, pb, c2b[:], s2[:], [P, TPC])
            nc.vector.tensor_tensor(out=nc2[:], in0=c2b[:], in1=npad2[:], op=ALU.mult)
            nc.vector.tensor_tensor(out=zc2[:], in0=nc2[:], in1=p2[:],
                                    op=ALU.subtract)

            # ---- layer 2 (dma_gather on 4 swdge queues) ----
            def head_group(g):
                nt = min(8, TPC - g * 8)
                ost = fin.tile([P, 8 * NCOUT], F32, tag="ost")
                for i in range(nt):
                    t = g * 8 + i
                    tp = auxp.tile([CW1, P], BF16, tag="ps_tp")
                    nc.tensor.transpose(tp[:C2, :], h3[:, t * C2:(t + 1) * C2], ident[:])
                    h3T = pa.tile([C2, P], BF16, tag="h3T")
                    nc.scalar.activation(h3T[:], tp[:C2, :], AF.Copy)
                    pso = psA.tile([P, 512], F32, tag="ps_small")
                    nc.tensor.matmul(pso[:, :NCOUT], lhsT=h3T[:], rhs=wf[:],
                                     start=True, stop=True)
                    nc.vector.tensor_tensor(out=ost[:, i * NCOUT:(i + 1) * NCOUT],
                                            in0=pso[:, :NCOUT], in1=bfr[:], op=ALU.add)
                dst_ap = t_out[g * 8 * P:g * 8 * P + nt * P, :].rearrange(
                    "(i p) c -> p i c", p=P)
                nc.sync.dma_start(dst_ap, ost[:, :nt * NCOUT].rearrange(
                    "p (i c) -> p i c", c=NCOUT))

            _layer2(nc, pg, fin, accp, meta, cfg, t_cc_out, t_idxg,
                    ad2, zc2, ident, o2, selfv2, rep2, nc2)
            _selu_epilogue(nc, ep, o2, b2r, o2, TPC, C2)
            h3 = o2
            for g in range((TPC + 7) // 8):
                head_group(g)

    nc.compile()
    return nc


def _lrelu_exp(nc, pool, out_ap, in_ap, shape):
    """exp(leakyrelu_0.2(s)) == max(exp(s), exp(0.2*s)) exactly."""
    e1 = pool.tile(shape, F32, tag="lre_r")
    nc.scalar.activation(e1[:], in_ap, AF.Exp)
    e2 = pool.tile(shape, F32, tag="lre_u")
    nc.scalar.activation(e2[:], in_ap, AF.Exp, scale=0.2)
    nc.vector.tensor_tensor(out=out_ap, in0=e1[:], in1=e2[:], op=ALU.max)


def _selu_epilogue(nc, ep, o, br, out_bf, TPC, CW, CHUNK=10):
    """out_bf = bf16(selu(o + b)), in CHUNK-tile batches (few big vector ops)."""
    for t0 in range(0, TPC, CHUNK):
        ntc = min(CHUNK, TPC - t0)
        _selu_chunk(nc, ep, o, br, out_bf, t0, ntc, CW, CHUNK)


def _selu_chunk(nc, ep, o, br, out_bf, t0, ntc, CW, CHUNK):
    if True:
        lo, hi = t0 * CW, (t0 + ntc) * CW
        n = ntc * CW
        vb = ep.tile([P, CHUNK * CW], F32, tag="selu_vb")
        nc.vector.tensor_tensor(out=vb[:, :n].rearrange("p (t c) -> p t c", c=CW),
                                in0=o[:, lo:hi].rearrange("p (t c) -> p t c", c=CW),
                                in1=br[:][:, None, :].to_broadcast([P, ntc, CW]),
                                op=ALU.add)
        rr = ep.tile([P, CHUNK * CW], F32, tag="selu_rr")
        nc.scalar.activation(rr[:, :n], vb[:, :n], AF.Relu)
        w = ep.tile([P, CHUNK * CW], F32, tag="selu_w")
        nc.vector.tensor_tensor(out=w[:, :n], in0=vb[:, :n], in1=rr[:, :n],
                                op=ALU.subtract)
        e = ep.tile([P, CHUNK * CW], F32, tag="selu_e")
        nc.scalar.activation(e[:, :n], w[:, :n], AF.Exp)
        nc.vector.tensor_scalar(out=e[:, :n], in0=e[:, :n], scalar1=SELU_ALPHA_SCALE,
                                scalar2=-SELU_ALPHA_SCALE, op0=ALU.mult, op1=ALU.add)
        nc.vector.tensor_scalar(out=rr[:, :n], in0=rr[:, :n], scalar1=SELU_SCALE,
                                scalar2=None, op0=ALU.mult)
        nc.vector.tensor_tensor(out=out_bf[:, lo:hi], in0=e[:, :n], in1=rr[:, :n],
                                op=ALU.add)


def _finalize_tile(nc, fin, out_res, acc, zc, selfv, toff, vrep, vmul, t, CW, H):
    z = fin.tile([P, H], F32, tag="zf")
    nc.vector.tensor_tensor(out=z[:], in0=acc[:, 0:H],
                            in1=zc[:, t * H:(t + 1) * H], op=ALU.subtract)
    nc.vector.tensor_scalar(out=z[:], in0=z[:], scalar1=1e-16, scalar2=None,
                            op0=ALU.add)
    nc.vector.reciprocal(z[:], z[:])
    avs = fin.tile([P, CW], F32, tag="avs")
    nc.vector.tensor_tensor(out=avs[:], in0=acc[:, H:],
                            in1=selfv[:, (t - toff) * CW:(t - toff + 1) * CW],
                            op=ALU.add)
    if vrep is not None:
        vc = fin.tile([P, CW], F32, tag="vc")
        nc.vector.tensor_tensor(out=vc[:], in0=vrep[:, 0:CW],
                                in1=vmul[:, t:t + 1].to_broadcast([P, CW]),
                                op=ALU.mult)
        nc.vector.tensor_tensor(out=avs[:], in0=avs[:], in1=vc[:],
                                op=ALU.subtract)
    nc.vector.tensor_tensor(
        out=out_res[:, t * CW:(t + 1) * CW].rearrange("p (h c) -> p h c", h=H),
        in0=avs[:].rearrange("p (h c) -> p h c", h=H),
        in1=z[:].to_broadcast([P, H, CW // H]),
        op=ALU.mult)


def _layer1(nc, pb, fin, accp, psA, groups, offs, t_xexpT, waug,
            F, CW, H, ad, zc, ident, out_res, t_selfv, progress=None):
    FV = H + CW
    copy_flip = [0]
    pending = []
    for (t0, nt, Jg) in groups:
        o = offs[t0]
        SJ = nt * Jg
        gath = pb.tile([P, SJ * F], BF16, tag="gath1")
        sv1t = pb.tile([P, 6 * CW], BF16, tag="sv1")
        nc.sync.dma_start(sv1t[:, :nt * CW], t_selfv[:, t0 * CW:(t0 + nt) * CW])
        xe = pb.tile([16, SJ * P], BF16, tag="xe")
        nc.sync.dma_start(xe[:], t_xexpT[:, o * P:(o + SJ) * P])
        BATCH = 5
        for s0 in range(0, SJ, BATCH):
            nb = min(BATCH, SJ - s0)
            psb = psA.tile([P, 512], F32, tag="ps_small")
            for i in range(nb):
                s = s0 + i
                nc.tensor.matmul(psb[:, i * F:i * F + F],
                                 lhsT=xe[:, s * P:(s + 1) * P],
                                 rhs=waug[:, :F], start=True, stop=True)
            if copy_flip[0] % 3 != 0:
                nc.scalar.activation(gath[:, s0 * F:(s0 + nb) * F],
                                     psb[:, :nb * F], AF.Copy)
            else:
                nc.vector.tensor_copy(gath[:, s0 * F:(s0 + nb) * F],
                                      psb[:, :nb * F])
            copy_flip[0] += 1
        gv = gath[:].rearrange("p (t j f) -> p t j f", j=Jg, f=F)
        s = pb.tile([P, SJ * H], F32, tag="s")
        s4 = s[:].rearrange("p (t j h) -> p t j h", j=Jg, h=H)
        adv = ad[:].rearrange("p (t h) -> p t h", h=H)[:, t0:t0 + nt, :]
        nc.vector.tensor_tensor(out=s4, in0=gv[:, :, :, CW:CW + H],
                                in1=adv[:, :, None, :].to_broadcast([P, nt, Jg, H]),
                                op=ALU.add)
        e1 = pb.tile([P, SJ * H], F32, tag="r")
        nc.scalar.activation(e1[:], s[:], AF.Exp)
        e2 = pb.tile([P, SJ * H], F32, tag="u")
        nc.scalar.activation(e2[:], s[:], AF.Exp, scale=0.2)
        rhs2 = pb.tile([P, SJ * FV], BF16, tag="rhs2")
        r2 = rhs2[:].rearrange("p (t j f) -> p t j f", j=Jg, f=FV)
        nc.vector.tensor_tensor(out=r2[:, :, :, 0:H],
                                in0=e1[:].rearrange("p (t j h) -> p t j h", j=Jg, h=H),
                                in1=e2[:].rearrange("p (t j h) -> p t j h", j=Jg, h=H),
                                op=ALU.max)
        nc.vector.tensor_tensor(
            out=r2[:, :, :, H:],
            in0=gv[:, :, :, 0:CW],
            in1=r2[:, :, :, 0:H].to_broadcast([P, nt, Jg, H, CW // H]),
            op=ALU.mult)
        accs = []
        for i in range(nt):
            t = t0 + i
            acc = accp.tile([P, FV], F32, tag="agg")
            for j in range(Jg):
                nc.tensor.matmul(acc[:], lhsT=ident[:],
                                 rhs=rhs2[:, (i * Jg + j) * FV:(i * Jg + j + 1) * FV],
                                 start=(j == 0), stop=(j == Jg - 1))
            accs.append((acc, t))
        for fn in pending:
            fn()
        svt, st0 = sv1t, t0
        pending = [
            (lambda a=a, tt=tt, svt=svt, st0=st0:
             _finalize_tile(nc, fin, out_res, a, zc, svt, st0, None, None,
                            tt, CW, H))
            for (a, tt) in accs]
        if progress is not None:
            progress(t0)
    for fn in pending:
        fn()
    if progress is not None:
        progress(10**9)


def _layer2(nc, pb, fin, accp, meta, cfg, t_cc, t_idxg, ad, zc, ident,
            out_res, selfv, vrep, vmul, progress=None):
    """dma_gather-based layer 2: grid [k(4)][tile][j] per group, H=1."""
    C2, CH = cfg.C2, cfg.CH
    F = TB2
    H = 1
    CW = C2
    FV = H + CW
    groups2, calls2, offs4 = meta["groups2"], meta["calls2"], meta["offs4"]
    pending = []
    for gi, (t0, nt, Jg) in enumerate(groups2):
        cols = nt * Jg
        SJ = NCHUNK * cols
        gath = pb.tile([P, SJ * F], BF16, tag="gath2")
        blk0 = calls2[gi][0][3]
        nblk_g = len(calls2[gi])
        idxt = pb.tile([P, 12 * 64], I16, tag="idxt")
        nc.sync.dma_start(idxt[:, :nblk_g * 64],
                          t_idxg[:, blk0 * 64:(blk0 + nblk_g) * 64])
        for (k, c0, ncol, blk) in calls2[gi]:
            out_ap = gath[:, (k * cols + c0) * F:(k * cols + c0 + ncol) * F]
            nc.gpsimd.dma_gather(
                out_ap=out_ap.rearrange("p (c e) -> p c e", e=F),
                in_ap=t_cc[k * CH:(k + 1) * CH, :],
                idxs_ap=idxt[:, (blk - blk0) * 64:(blk - blk0) * 64 + ncol * 8],
                num_idxs=ncol * P,
                num_idxs_reg=ncol * P,
                elem_size=F,
                queue_num=k,
            )
        # views: [p, k, t, j, f]
        gv = gath[:].rearrange("p (k t j f) -> p k t j f", k=NCHUNK, j=Jg, f=F)
        s = pb.tile([P, SJ], F32, tag="s")
        s5 = s[:].rearrange("p (k t j) -> p k t j", k=NCHUNK, j=Jg)
        adv = ad[:, t0:t0 + nt]
        nc.vector.tensor_tensor(
            out=s5, in0=gv[:, :, :, :, CW:CW + 1],
            in1=adv[:, None, :, None].to_broadcast([P, NCHUNK, nt, Jg]),
            op=ALU.add)
        e1 = pb.tile([P, SJ], F32, tag="r")
        nc.scalar.activation(e1[:], s[:], AF.Exp)
        e2 = pb.tile([P, SJ], F32, tag="u")
        nc.scalar.activation(e2[:], s[:], AF.Exp, scale=0.2)
        rhs2 = pb.tile([P, SJ * FV], BF16, tag="rhs2")
        r2 = rhs2[:].rearrange("p (k t j f) -> p k t j f", k=NCHUNK, j=Jg, f=FV)
        nc.vector.tensor_tensor(out=r2[:, :, :, :, 0:H],
                                in0=e1[:].rearrange("p (k t j) -> p k t j",
                                                    k=NCHUNK, j=Jg),
                                in1=e2[:].rearrange("p (k t j) -> p k t j",
                                                    k=NCHUNK, j=Jg),
                                op=ALU.max)
        nc.vector.tensor_tensor(
            out=r2[:, :, :, :, H:],
            in0=gv[:, :, :, :, 0:CW],
            in1=r2[:, :, :, :, 0:H].to_broadcast([P, NCHUNK, nt, Jg, H, CW]),
            op=ALU.mult)
        accs = []
        for i in range(nt):
            t = t0 + i
            acc = accp.tile([P, FV], F32, tag="agg")
            nmm = NCHUNK * Jg
            mi = 0
            for k in range(NCHUNK):
                for j in range(Jg):
                    sl = ((k * nt + i) * Jg + j) * FV
                    nc.tensor.matmul(acc[:], lhsT=ident[:], rhs=rhs2[:, sl:sl + FV],
                                     start=(mi == 0), stop=(mi == nmm - 1))
                    mi += 1
            accs.append((acc, t))
        for fn in pending:
            fn()
        pending = [
            (lambda a=a, tt=tt:
             _finalize_tile(nc, fin, out_res, a, zc, selfv, 0, vrep, vmul,
                            tt, CW, H))
            for (a, tt) in accs]
        if progress is not None:
            progress(t0)
    for fn in pending:
        fn()
    if progress is not None:
        progress(10**9)


def _make_inputs(cfg, percore, waug1, waug2, inputs):
    wf = np.asarray(inputs["Wf"], np.float32).astype(NPBF16)
    b1r = np.broadcast_to(np.asarray(inputs["b1"], np.float32), (P, cfg.CW1)).copy()
    b2r = np.broadcast_to(np.asarray(inputs["b2"], np.float32), (P, cfg.C2)).copy()
    bfr = np.broadcast_to(np.asarray(inputs["bf"], np.float32), (P, cfg.NCOUT)).copy()
    waug1_bf = waug1.astype(NPBF16)
    waug2_bf = waug2.astype(NPBF16)
    in_maps = []
    for c in range(cfg.ncores):
        pc = percore[c]
        in_maps.append({
            "xexpT": pc["xexpT"], "selfv1": pc["selfv1"], "idxg": pc["idxg"],
            "waug1": waug1_bf, "waug2": waug2_bf, "wf": wf,
            "npad2": pc["npad2"], "ad1": pc["ad1"], "zc1": pc["zc1"],
            "b1r": b1r, "b2r": b2r, "bfr": bfr,
        })
    return in_maps


def _assemble(cfg, percore, results):
    out = np.zeros((cfg.NPAD, cfg.NCOUT), np.float32)
    for c in range(cfg.ncores):
        out[percore[c]["own"]] = results[c]["out"]
    return out[:cfg.N]


def kernel(**inputs) -> np.ndarray:
    cfg = Cfg(N=100000, E=800000, ncores=8)
    percore, waug1, waug2, meta = preprocess(
        cfg,
        np.asarray(inputs["x"], np.float32),
        np.asarray(inputs["edge_index"]),
        np.asarray(inputs["W1"], np.float32),
        np.asarray(inputs["a_src1"], np.float32),
        np.asarray(inputs["a_dst1"], np.float32),
        np.asarray(inputs["W2"], np.float32),
        np.asarray(inputs["a_src2"], np.float32),
        np.asarray(inputs["a_dst2"], np.float32),
    )
    nc = build_nc(cfg, meta)
    in_maps = _make_inputs(cfg, percore, waug1, waug2, inputs)
    res = run_bass_kernel_spmd(nc, in_maps, core_ids=list(range(cfg.ncores)))
    return _assemble(cfg, percore, res.results)


if __name__ == "__main__":
    import reference as R
    inputs = R.setup_inputs()
    out = kernel(**{k: np.asarray(v) for k, v in inputs.items()})
    print("out", out.shape, out.dtype)


# revision 30
# speedup vs baseline: 1.0380x; 1.0380x over previous
"""Trainium2 Bass kernel for nn_GAT_1675037246077 (2-layer GAT + linear head).

Strategy (8 NeuronCores, SPMD single NEFF):
 - Destination-sharded: core c owns dst nodes [c*12544, (c+1)*12544); nodes padded
   to NPAD = 100352 = 8*128*98 (pad nodes x=0, degree 0). Per-core nodes sorted by
   max-per-chunk in-degree (desc) and packed into 98 tiles of 128 nodes.
 - Edge grids EXCLUDE self-loops (self terms use resident own-node data).
 - Layer 1: host-expanded x in edge-slot order -> per-slot [16,128]@[16,99]
   matmuls (5 slots share a PSUM tile + one copy). Grid [128, J1_t] per tile.
 - Layer 2: table2[NPAD, 128] = [h2(32)|as2(1)|0...] built shard-wise,
   AllGather'd, then gathered per edge via InstDMAGatherAnt (dma_gather) on 4
   SWDGE queues: 256B rows, int16 indices local to 4 source-position chunks of
   25088 rows. Grid per tile: 4 chunk-pure segments of J4u_t columns each
   (chunk-major within group); <=1024 indices per call (descriptor carveout).
 - Scores exp(leakyrelu(s)) = exp(0.2*(4*relu(s)+s)); weighted aggregation via
   identity-matmul PSUM accumulation; exact pad-slot corrections (all pad rows
   have identical table2 content); softmax divide; bias+selu chunked epilogue.
"""
import numpy as np
import ml_dtypes

from concourse import mybir, tile, bacc
import concourse.bass as bass
from concourse.bass_utils import run_bass_kernel_spmd
from concourse.masks import make_identity

P = 128
AF = mybir.ActivationFunctionType
ALU = mybir.AluOpType
BF16 = mybir.dt.bfloat16
F32 = mybir.dt.float32
I16 = mybir.dt.int16
NPBF16 = ml_dtypes.bfloat16

SELU_SCALE = 1.0507009873554805
SELU_ALPHA_SCALE = 1.7580993408473766

NCHUNK = 4
TB2 = 128          # padded table2 row (bf16 cols) = 256B
MAXC = 8           # max columns per dma_gather call (1024 idxs)


class Cfg:
    def __init__(self, N, E, ncores, fin=16, h1=3, c1=32, c2=32, ncout=16):
        self.N, self.E, self.ncores = N, E, ncores
        self.FIN, self.H1, self.C1, self.C2, self.NCOUT = fin, h1, c1, c2, ncout
        self.CW1 = h1 * c1              # 96
        self.F1 = self.CW1 + h1         # table1 cols (99)
        self.SH = ((N + ncores - 1) // ncores + P - 1) // P * P
        self.TPC = self.SH // P
        self.NPAD = self.SH * ncores
        self.CH = self.NPAD // NCHUNK   # 25088
        self.WAUG2_C = c2 + 2           # [W2(32)|as2|ad2]


def _group_plan(J, cap, max_nt):
    groups = []
    t = 0
    n = len(J)
    while t < n:
        j = J[t]
        nt = 1
        while t + nt < n and J[t + nt] == j and (nt + 1) * j <= cap and nt < max_nt:
            nt += 1
        groups.append((t, nt, int(j)))
        t += nt
    return groups


def preprocess(cfg, x, edge_index, W1, a_src1, a_dst1, W2, a_src2, a_dst2):
    N, E, NC = cfg.N, cfg.E, cfg.ncores
    SH, TPC, NPAD, CH = cfg.SH, cfg.TPC, cfg.NPAD, cfg.CH

    src = edge_index[0].astype(np.int64)
    dst = edge_index[1].astype(np.int64)
    deg = np.bincount(dst, minlength=NPAD)

    # interleaved sharding: node n -> core n % NC (spreads pad nodes to all
    # cores so every chunk's last position is a zero pad row).
    src_chunk = (src % NC) // 2
    cnt = np.zeros((NPAD, NCHUNK), np.int64)
    np.add.at(cnt, (dst, src_chunk), 1)
    mx = cnt.max(1)
    key = -(mx * 1000000 + deg)

    perms = []
    pos = np.empty(NPAD, np.int64)
    J1_all = np.zeros((NC, TPC), np.int64)
    J4_all = np.zeros((NC, TPC), np.int64)
    for c in range(NC):
        ids = np.arange(c, NPAD, NC)
        order = np.argsort(key[ids], kind="stable")
        perm = ids[order]
        perms.append(perm)
        pos[perm] = c * SH + np.arange(SH)
        pc = cnt[perm].reshape(TPC, P, NCHUNK)
        J1_all[c] = pc.sum(axis=2).max(axis=1)
        J4_all[c] = pc.max(axis=1).max(axis=1)
    J1 = np.maximum(J1_all.max(0), 1)
    J4u = np.maximum(J4_all.max(0), 1)
    SLOTS = int(J1.sum())
    NC4 = int(J4u.sum())        # columns per chunk in the L2 grid
    offs1 = np.zeros(TPC + 1, np.int64)
    np.cumsum(J1, out=offs1[1:])
    offs4 = np.zeros(TPC + 1, np.int64)
    np.cumsum(J4u, out=offs4[1:])

    e_order = np.argsort(dst, kind="stable")
    src_sorted = src[e_order]
    starts = np.zeros(NPAD + 1, np.int64)
    np.cumsum(deg, out=starts[1:])

    FIN, H1, C1, CW1 = cfg.FIN, cfg.H1, cfg.C1, cfg.CW1
    waug1 = np.zeros((FIN, CW1 + 2 * H1), np.float32)
    waug1[:, :CW1] = W1
    for h in range(H1):
        waug1[:, CW1 + h] = W1[:, h * C1:(h + 1) * C1] @ a_src1[h]
        waug1[:, CW1 + H1 + h] = W1[:, h * C1:(h + 1) * C1] @ a_dst1[h]
    C2 = cfg.C2
    waug2 = np.zeros((CW1, cfg.WAUG2_C), np.float32)
    waug2[:, :C2] = W2
    waug2[:, C2] = W2[:, :] @ a_src2[0]
    waug2[:, C2 + 1] = W2[:, :] @ a_dst2[0]

    xpad = np.zeros((NPAD, FIN), np.float32)
    xpad[:N] = x
    xw = xpad @ waug1
    xpadT_bf = np.ascontiguousarray(xpad.T).astype(NPBF16)

    # L2 group + call plan (shared across cores; SPMD constants)
    groups2 = _group_plan(list(J4u), 16, 8)
    calls2 = []   # per group: list of (k, c0, nc, blk)
    nblk = 0
    for (t0, nt, Jg) in groups2:
        cl = []
        cols = nt * Jg
        for k in range(NCHUNK):
            c0 = 0
            while c0 < cols:
                ncol = min(MAXC, cols - c0)
                cl.append((k, c0, ncol, nblk))
                nblk += 1
                c0 += ncol
        calls2.append(cl)

    def pslice(vals, w):
        return np.ascontiguousarray(
            vals.reshape(TPC, P, w).transpose(1, 0, 2).reshape(P, TPC * w))

    def bf(v):
        return v.astype(NPBF16).astype(np.float32)

    percore = []
    for c in range(NC):
        perm = perms[c]
        idx1 = np.full((P, SLOTS), NPAD - 1, np.int32)
        npad2 = np.zeros((P, TPC), np.float32)
        idxL2 = np.full((NCHUNK, P, NC4), CH - 1, np.int64)
        for t in range(TPC):
            jt = int(J1[t])
            o1 = int(offs1[t])
            o4 = int(offs4[t])
            for p in range(P):
                node = perm[t * P + p]
                dg = int(deg[node])
                s0 = int(starts[node])
                ss = src_sorted[s0:s0 + dg]
                idx1[p, o1:o1 + dg] = ss
                sp = pos[ss]
                ck = sp // CH
                for k in range(NCHUNK):
                    sel = sp[ck == k] - k * CH
                    idxL2[k, p, o4:o4 + len(sel)] = sel
            npad2[:, t] = NCHUNK * int(J4u[t])
        # subtract real degree to get pad counts
        dgp = deg[perm].reshape(TPC, P).T  # [P, TPC]
        npad2 -= dgp
        # wrapped int16 idx blocks, in call order
        idxg = np.zeros((16, nblk * 64), np.int16)
        for gi, (t0, nt, Jg) in enumerate(groups2):
            for (k, c0, ncol, blk) in calls2[gi]:
                g0 = int(offs4[t0]) + c0
                seq = idxL2[k][:, g0:g0 + ncol].T.ravel()  # p fastest
                w = seq.reshape(-1, 16).T.astype(np.int16)  # [16, ncol*8]
                idxg[:, blk * 64: blk * 64 + ncol * 8] = w
        idxg_full = np.tile(idxg, (8, 1))

        cols1 = idx1.T.ravel()
        xexpT = np.ascontiguousarray(xpadT_bf[:, cols1])
        ad1v = xw[perm, CW1 + H1:CW1 + 2 * H1]
        as1v = xw[perm, CW1:CW1 + H1]
        sv = (as1v + ad1v).astype(np.float32)
        p1 = bf(np.maximum(np.exp(sv), np.exp(0.2 * sv)))
        a1f = ad1v.astype(np.float32)
        c1 = bf(np.maximum(np.exp(a1f), np.exp(0.2 * a1f)))
        npad1 = np.zeros((P, TPC), np.float32)
        npad1[:] = J1[None, :].astype(np.float32)
        npad1 -= dgp
        zc1 = (npad1[:, :, None] * pslice(c1, H1).reshape(P, TPC, H1)
               - pslice(p1, H1).reshape(P, TPC, H1)).reshape(P, TPC * H1)
        h1v = xw[perm, :CW1]
        selfv1 = pslice((p1[:, :, None] * h1v.reshape(SH, H1, C1)).reshape(SH, CW1),
                        CW1)
        percore.append(dict(
            npad2=npad2, own=perm.astype(np.int64),
            xexpT=xexpT, idxg=idxg_full,
            ad1=pslice(ad1v, H1).astype(np.float32),
            zc1=np.ascontiguousarray(zc1, np.float32),
            selfv1=selfv1.astype(NPBF16)))

    meta = dict(J1=[int(j) for j in J1], offs1=[int(o) for o in offs1],
                J4u=[int(j) for j in J4u], offs4=[int(o) for o in offs4],
                SLOTS=SLOTS, NC4=NC4, groups2=groups2, calls2=calls2, nblk=nblk)
    return percore, waug1, waug2, meta


def build_nc(cfg, meta, debug=False):
    J1, offs1 = meta["J1"], meta["offs1"]
    SLOTS, nblk = meta["SLOTS"], meta["nblk"]
    groups2, calls2 = meta["groups2"], meta["calls2"]
    TPC, NPAD, SH, CH = cfg.TPC, cfg.NPAD, cfg.SH, cfg.CH
    FIN, H1, CW1, C2, F1 = cfg.FIN, cfg.H1, cfg.CW1, cfg.C2, cfg.F1
    NCOUT = cfg.NCOUT
    groups1 = _group_plan(J1, 48, 6)

    nc = bacc.Bacc("TRN2", target_bir_lowering=False, debug=debug,
                   num_devices=cfg.ncores, num_swdge_queues=4)

    t_xexpT = nc.dram_tensor("xexpT", [FIN, SLOTS * P], BF16, kind="ExternalInput")
    t_selfv1 = nc.dram_tensor("selfv1", [P, TPC * CW1], BF16, kind="ExternalInput")
    t_idxg = nc.dram_tensor("idxg", [P, nblk * 64], I16, kind="ExternalInput")
    t_waug1 = nc.dram_tensor("waug1", [FIN, CW1 + 2 * H1], BF16, kind="ExternalInput")
    t_waug2 = nc.dram_tensor("waug2", [CW1, cfg.WAUG2_C], BF16, kind="ExternalInput")
    t_wf = nc.dram_tensor("wf", [C2, NCOUT], BF16, kind="ExternalInput")
    t_npad2 = nc.dram_tensor("npad2", [P, TPC], F32, kind="ExternalInput")
    t_ad1 = nc.dram_tensor("ad1", [P, TPC * H1], F32, kind="ExternalInput")
    t_zc1 = nc.dram_tensor("zc1", [P, TPC * H1], F32, kind="ExternalInput")
    t_b1r = nc.dram_tensor("b1r", [P, CW1], F32, kind="ExternalInput")
    t_b2r = nc.dram_tensor("b2r", [P, C2], F32, kind="ExternalInput")
    t_bfr = nc.dram_tensor("bfr", [P, NCOUT], F32, kind="ExternalInput")
    t_out = nc.dram_tensor("out", [SH, NCOUT], F32, kind="ExternalOutput")

    t_cc_in = nc.dram_tensor("cc_in", [SH, TB2], BF16)
    cc_space = "Shared" if cfg.ncores > 4 else "Local"
    t_cc_out = nc.dram_tensor("cc_out", [NPAD, TB2], BF16, addr_space=cc_space)

    with tile.TileContext(nc) as tc:
        with (
            tc.tile_pool(name="res", bufs=1) as res,
            tc.tile_pool(name="pa", bufs=3) as pa,
            tc.tile_pool(name="pb", bufs=2) as pb,
            tc.tile_pool(name="pg", bufs=3) as pg,
            tc.tile_pool(name="fin", bufs=2) as fin,
            tc.tile_pool(name="ep", bufs=1) as ep,
            tc.tile_pool(name="psA", bufs=2, space="PSUM") as psA,
            tc.tile_pool(name="acc", bufs=4, space="PSUM") as accp,
            tc.tile_pool(name="aux", bufs=2, space="PSUM") as auxp,
        ):
            ident = res.tile([P, P], BF16)
            make_identity(nc, ident[:])
            waug1 = res.tile([FIN, CW1 + 2 * H1], BF16)
            nc.sync.dma_start(waug1[:], t_waug1[:, :])
            waug2 = res.tile([CW1, cfg.WAUG2_C], BF16)
            nc.sync.dma_start(waug2[:], t_waug2[:, :])
            wf = res.tile([C2, NCOUT], BF16)
            nc.sync.dma_start(wf[:], t_wf[:, :])
            npad2 = res.tile([P, TPC], F32)
            nc.sync.dma_start(npad2[:], t_npad2[:, :])
            b1r = res.tile([P, CW1], F32)
            nc.sync.dma_start(b1r[:], t_b1r[:, :])
            b2r = res.tile([P, C2], F32)
            nc.sync.dma_start(b2r[:], t_b2r[:, :])
            bfr = res.tile([P, NCOUT], F32)
            nc.sync.dma_start(bfr[:], t_bfr[:, :])
            ad1 = res.tile([P, TPC * H1], F32)
            nc.sync.dma_start(ad1[:], t_ad1[:, :])
            zc1 = res.tile([P, TPC * H1], F32)
            nc.sync.dma_start(zc1[:], t_zc1[:, :])
            ad2 = res.tile([P, TPC], F32)
            hoa = res.tile([P, TPC * (C2 + 1)], BF16)  # [h2own(32)|as2own]
            zc2 = res.tile([P, TPC], F32)
            nc2 = res.tile([P, TPC], F32)
            o1 = res.tile([P, TPC * CW1], BF16)   # pre-selu; reused as h2in
            o2 = res.tile([P, TPC * C2], BF16)    # pre-selu; reused as h3

            # ---- layer 1 (mm expansion) ----
            _layer1(nc, pb, fin, accp, psA, groups1, offs1, t_xexpT, waug1,
                    F1, CW1, H1, ad1, zc1, ident, o1, t_selfv1)
            _selu_epilogue(nc, ep, o1, b1r, o1, TPC, CW1)
            h2in = o1

            # ---- table2 build + AllGather ----
            def build_t2(t):
                tp = auxp.tile([CW1, P], BF16, tag="ps_tp")
                nc.tensor.transpose(tp[:], h2in[:, t * CW1:(t + 1) * CW1], ident[:])
                h2T = pa.tile([CW1, P], BF16, tag="h2T")
                nc.scalar.activation(h2T[:], tp[:], AF.Copy)
                ps2 = psA.tile([P, 512], F32, tag="ps_small")
                nc.tensor.matmul(ps2[:, :cfg.WAUG2_C], lhsT=h2T[:], rhs=waug2[:],
                                 start=True, stop=True)
                nc.vector.tensor_copy(ad2[:, t:t + 1], ps2[:, C2 + 1:C2 + 2])
                nc.scalar.activation(hoa[:, t * (C2 + 1):(t + 1) * (C2 + 1)],
                                     ps2[:, :C2 + 1], AF.Copy)
                st2 = pa.tile([P, C2 + 1], BF16, tag="st2")
                nc.scalar.activation(st2[:], ps2[:, :C2 + 1], AF.Copy)
                nc.sync.dma_start(t_cc_in[t * P:(t + 1) * P, 0:C2 + 1], st2[:])

            for t in range(TPC):
                build_t2(t)

            s2p = pb.tile([P, TPC], F32, tag="s2c")
            hoav = hoa[:].rearrange("p (t k) -> p t k", k=C2 + 1)
            nc.vector.tensor_tensor(out=s2p[:][:, :, None], in0=hoav[:, :, C2:C2 + 1],
                                    in1=ad2[:][:, :, None], op=ALU.add)
            p2 = res.tile([P, TPC], BF16)
            _lrelu_exp(nc, pb, p2[:], s2p[:], [P, TPC])
            selfv2 = res.tile([P, TPC * C2], BF16)
            nc.vector.tensor_tensor(
                out=selfv2[:].rearrange("p (t c) -> p t c", c=C2),
                in0=hoav[:, :, 0:C2],
                in1=p2[:][:, :, None].to_broadcast([P, TPC, C2]),
                op=ALU.mult)
            nc.gpsimd.collective_compute(
                "AllGather", ALU.bypass,
                replica_groups=[list(range(cfg.ncores))],
                ins=[t_cc_in.ap().opt()],
                outs=[t_cc_out.ap().opt()],
            )

            rep2 = res.tile([P, C2 + 1], F32)
            rep_src = bass.AP(tensor=t_cc_out.ap().tensor, offset=(NPAD - 1) * TB2,
                              ap=[[0, P], [1, C2 + 1]])
            nc.gpsimd.dma_start(out=rep2[:], in_=rep_src)

            s2 = pb.tile([P, TPC], F32, tag="s2c")
            nc.vector.tensor_tensor(out=s2[:], in0=ad2[:],
                                    in1=rep2[:, C2:C2 + 1].to_broadcast([P, TPC]),
                                    op=ALU.add)
            c2b = res.tile([P, TPC], BF16)
            _lrelu_exp(nc, pb, c2b[:], s2[:], [P, TPC])
            nc.vector.tensor_tensor(out=nc2[:], in0=c2b[:], in1=npad2[:], op=ALU.mult)
            nc.vector.tensor_tensor(out=zc2[:], in0=nc2[:], in1=p2[:],
                                    op=ALU.subtract)

            # ---- layer 2 (dma_gather on 4 swdge queues) ----
            def head_group(g):
                nt = min(8, TPC - g * 8)
                ost = fin.tile([P, 8 * NCOUT], F32, tag="ost")
                for i in range(nt):
                    t = g * 8 + i
                    tp = auxp.tile([CW1, P], BF16, tag="ps_tp")
                    nc.tensor.transpose(tp[:C2, :], h3[:, t * C2:(t + 1) * C2], ident[:])
                    h3T = pa.tile([C2, P], BF16, tag="h3T")
                    nc.scalar.activation(h3T[:], tp[:C2, :], AF.Copy)
                    pso = psA.tile([P, 512], F32, tag="ps_small")
                    nc.tensor.matmul(pso[:, :NCOUT], lhsT=h3T[:], rhs=wf[:],
                                     start=True, stop=True)
                    nc.vector.tensor_tensor(out=ost[:, i * NCOUT:(i + 1) * NCOUT],
                                            in0=pso[:, :NCOUT], in1=bfr[:], op=ALU.add)
                dst_ap = t_out[g * 8 * P:g * 8 * P + nt * P, :].rearrange(
                    "(i p) c -> p i c", p=P)
                nc.sync.dma_start(dst_ap, ost[:, :nt * NCOUT].rearrange(
                    "p (i c) -> p i c", c=NCOUT))

            _layer2(nc, pg, fin, accp, meta, cfg, t_cc_out, t_idxg,
                    ad2, zc2, ident, o2, selfv2, rep2, nc2)
            _selu_epilogue(nc, ep, o2, b2r, o2, TPC, C2)
            h3 = o2
            for g in range((TPC + 7) // 8):
                head_group(g)

    nc.compile()
    return nc


def _lrelu_exp(nc, pool, out_ap, in_ap, shape):
    """exp(leakyrelu_0.2(s)) == max(exp(s), exp(0.2*s)) exactly."""
    e1 = pool.tile(shape, F32, tag="lre_r")
    nc.scalar.activation(e1[:], in_ap, AF.Exp)
    e2 = pool.tile(shape, F32, tag="lre_u")
    nc.scalar.activation(e2[:], in_ap, AF.Exp, scale=0.2)
    nc.vector.tensor_tensor(out=out_ap, in0=e1[:], in1=e2[:], op=ALU.max)


def _selu_epilogue(nc, ep, o, br, out_bf, TPC, CW, CHUNK=10):
    """out_bf = bf16(selu(o + b)), in CHUNK-tile batches (few big vector ops)."""
    for t0 in range(0, TPC, CHUNK):
        ntc = min(CHUNK, TPC - t0)
        _selu_chunk(nc, ep, o, br, out_bf, t0, ntc, CW, CHUNK)


def _selu_chunk(nc, ep, o, br, out_bf, t0, ntc, CW, CHUNK):
    if True:
        lo, hi = t0 * CW, (t0 + ntc) * CW
        n = ntc * CW
        vb = ep.tile([P, CHUNK * CW], F32, tag="selu_vb")
        nc.vector.tensor_tensor(out=vb[:, :n].rearrange("p (t c) -> p t c", c=CW),
                                in0=o[:, lo:hi].rearrange("p (t c) -> p t c", c=CW),
                                in1=br[:][:, None, :].to_broadcast([P, ntc, CW]),
                                op=ALU.add)
        rr = ep.tile([P, CHUNK * CW], F32, tag="selu_rr")
        nc.scalar.activation(rr[:, :n], vb[:, :n], AF.Relu)
        w = ep.tile([P, CHUNK * CW], F32, tag="selu_w")
        nc.vector.tensor_tensor(out=w[:, :n], in0=vb[:, :n], in1=rr[:, :n],
                                op=ALU.subtract)
        e = ep.tile([P, CHUNK * CW], F32, tag="selu_e")
        nc.scalar.activation(e[:, :n], w[:, :n], AF.Exp)
        nc.vector.tensor_scalar(out=e[:, :n], in0=e[:, :n], scalar1=SELU_ALPHA_SCALE,
                                scalar2=-SELU_ALPHA_SCALE, op0=ALU.mult, op1=ALU.add)
        nc.vector.tensor_scalar(out=rr[:, :n], in0=rr[:, :n], scalar1=SELU_SCALE,
                                scalar2=None, op0=ALU.mult)
        nc.vector.tensor_tensor(out=out_bf[:, lo:hi], in0=e[:, :n], in1=rr[:, :n],
                                op=ALU.add)


def _finalize_tile(nc, fin, out_res, acc, zc, selfv, toff, vrep, vmul, t, CW, H):
    z = fin.tile([P, H], F32, tag="zf")
    nc.vector.tensor_tensor(out=z[:], in0=acc[:, 0:H],
                            in1=zc[:, t * H:(t + 1) * H], op=ALU.subtract)
    nc.vector.tensor_scalar(out=z[:], in0=z[:], scalar1=1e-16, scalar2=None,
                            op0=ALU.add)
    nc.vector.reciprocal(z[:], z[:])
    avs = fin.tile([P, CW], F32, tag="avs")
    nc.vector.tensor_tensor(out=avs[:], in0=acc[:, H:],
                            in1=selfv[:, (t - toff) * CW:(t - toff + 1) * CW],
                            op=ALU.add)
    if vrep is not None:
        vc = fin.tile([P, CW], F32, tag="vc")
        nc.vector.tensor_tensor(out=vc[:], in0=vrep[:, 0:CW],
                                in1=vmul[:, t:t + 1].to_broadcast([P, CW]),
                                op=ALU.mult)
        nc.vector.tensor_tensor(out=avs[:], in0=avs[:], in1=vc[:],
                                op=ALU.subtract)
    nc.vector.tensor_tensor(
        out=out_res[:, t * CW:(t + 1) * CW].rearrange("p (h c) -> p h c", h=H),
        in0=avs[:].rearrange("p (h c) -> p h c", h=H),
        in1=z[:].to_broadcast([P, H, CW // H]),
        op=ALU.mult)


def _layer1(nc, pb, fin, accp, psA, groups, offs, t_xexpT, waug,
            F, CW, H, ad, zc, ident, out_res, t_selfv, progress=None):
    FV = H + CW
    copy_flip = [0]
    for (t0, nt, Jg) in groups:
        o = offs[t0]
        SJ = nt * Jg
        gath = pb.tile([P, SJ * F], BF16, tag="gath1")
        sv1t = pb.tile([P, 6 * CW], BF16, tag="sv1")
        nc.sync.dma_start(sv1t[:, :nt * CW], t_selfv[:, t0 * CW:(t0 + nt) * CW])
        xe = pb.tile([16, SJ * P], BF16, tag="xe")
        nc.sync.dma_start(xe[:], t_xexpT[:, o * P:(o + SJ) * P])
        BATCH = 5
        for s0 in range(0, SJ, BATCH):
            nb = min(BATCH, SJ - s0)
            psb = psA.tile([P, 512], F32, tag="ps_small")
            for i in range(nb):
                s = s0 + i
                nc.tensor.matmul(psb[:, i * F:i * F + F],
                                 lhsT=xe[:, s * P:(s + 1) * P],
                                 rhs=waug[:, :F], start=True, stop=True)
            if copy_flip[0] % 3 != 0:
                nc.scalar.activation(gath[:, s0 * F:(s0 + nb) * F],
                                     psb[:, :nb * F], AF.Copy)
            else:
                nc.vector.tensor_copy(gath[:, s0 * F:(s0 + nb) * F],
                                      psb[:, :nb * F])
            copy_flip[0] += 1
        gv = gath[:].rearrange("p (t j f) -> p t j f", j=Jg, f=F)
        s = pb.tile([P, SJ * H], F32, tag="s")
        s4 = s[:].rearrange("p (t j h) -> p t j h", j=Jg, h=H)
        adv = ad[:].rearrange("p (t h) -> p t h", h=H)[:, t0:t0 + nt, :]
        nc.vector.tensor_tensor(out=s4, in0=gv[:, :, :, CW:CW + H],
                                in1=adv[:, :, None, :].to_broadcast([P, nt, Jg, H]),
                                op=ALU.add)
        e1 = pb.tile([P, SJ * H], F32, tag="r")
        nc.scalar.activation(e1[:], s[:], AF.Exp)
        e2 = pb.tile([P, SJ * H], F32, tag="u")
        nc.scalar.activation(e2[:], s[:], AF.Exp, scale=0.2)
        rhs2 = pb.tile([P, SJ * FV], BF16, tag="rhs2")
        r2 = rhs2[:].rearrange("p (t j f) -> p t j f", j=Jg, f=FV)
        nc.vector.tensor_tensor(out=r2[:, :, :, 0:H],
                                in0=e1[:].rearrange("p (t j h) -> p t j h", j=Jg, h=H),
                                in1=e2[:].rearrange("p (t j h) -> p t j h", j=Jg, h=H),
                                op=ALU.max)
        nc.vector.tensor_tensor(
            out=r2[:, :, :, H:],
            in0=gv[:, :, :, 0:CW],
            in1=r2[:, :, :, 0:H].to_broadcast([P, nt, Jg, H, CW // H]),
            op=ALU.mult)
        for i in range(nt):
            t = t0 + i
            acc = accp.tile([P, FV], F32, tag="agg")
            for j in range(Jg):
                nc.tensor.matmul(acc[:], lhsT=ident[:],
                                 rhs=rhs2[:, (i * Jg + j) * FV:(i * Jg + j + 1) * FV],
                                 start=(j == 0), stop=(j == Jg - 1))
            _finalize_tile(nc, fin, out_res, acc, zc, sv1t, t0, None, None, t, CW, H)
        if progress is not None:
            progress(t0 + nt)


def _layer2(nc, pb, fin, accp, meta, cfg, t_cc, t_idxg, ad, zc, ident,
            out_res, selfv, vrep, vmul, progress=None):
    """dma_gather-based layer 2: grid [k(4)][tile][j] per group, H=1."""
    C2, CH = cfg.C2, cfg.CH
    F = TB2
    H = 1
    CW = C2
    FV = H + CW
    groups2, calls2, offs4 = meta["groups2"], meta["calls2"], meta["offs4"]
    for gi, (t0, nt, Jg) in enumerate(groups2):
        cols = nt * Jg
        SJ = NCHUNK * cols
        gath = pb.tile([P, SJ * F], BF16, tag="gath2")
        blk0 = calls2[gi][0][3]
        nblk_g = len(calls2[gi])
        idxt = pb.tile([P, 12 * 64], I16, tag="idxt")
        nc.sync.dma_start(idxt[:, :nblk_g * 64],
                          t_idxg[:, blk0 * 64:(blk0 + nblk_g) * 64])
        for (k, c0, ncol, blk) in calls2[gi]:
            out_ap = gath[:, (k * cols + c0) * F:(k * cols + c0 + ncol) * F]
            nc.gpsimd.dma_gather(
                out_ap=out_ap.rearrange("p (c e) -> p c e", e=F),
                in_ap=t_cc[k * CH:(k + 1) * CH, :],
                idxs_ap=idxt[:, (blk - blk0) * 64:(blk - blk0) * 64 + ncol * 8],
                num_idxs=ncol * P,
                num_idxs_reg=ncol * P,
                elem_size=F,
                queue_num=k,
            )
        # views: [p, k, t, j, f]
        gv = gath[:].rearrange("p (k t j f) -> p k t j f", k=NCHUNK, j=Jg, f=F)
        s = pb.tile([P, SJ], F32, tag="s")
        s5 = s[:].rearrange("p (k t j) -> p k t j", k=NCHUNK, j=Jg)
        adv = ad[:, t0:t0 + nt]
        nc.vector.tensor_tensor(
            out=s5, in0=gv[:, :, :, :, CW:CW + 1],
            in1=adv[:, None, :, None].to_broadcast([P, NCHUNK, nt, Jg]),
            op=ALU.add)
        e1 = pb.tile([P, SJ], F32, tag="r")
        nc.scalar.activation(e1[:], s[:], AF.Exp)
        e2 = pb.tile([P, SJ], F32, tag="u")
        nc.scalar.activation(e2[:], s[:], AF.Exp, scale=0.2)
        rhs2 = pb.tile([P, SJ * FV], BF16, tag="rhs2")
        r2 = rhs2[:].rearrange("p (k t j f) -> p k t j f", k=NCHUNK, j=Jg, f=FV)
        nc.vector.tensor_tensor(out=r2[:, :, :, :, 0:H],
                                in0=e1[:].rearrange("p (k t j) -> p k t j",
                                                    k=NCHUNK, j=Jg),
                                in1=e2[:].rearrange("p (k t j) -> p k t j",
                                                    k=NCHUNK, j=Jg),
                                op=ALU.max)
        nc.vector.tensor_tensor(
            out=r2[:, :, :, :, H:],
            in0=gv[:, :, :, :, 0:CW],
            in1=r2[:, :, :, :, 0:H].to_broadcast([P, NCHUNK, nt, Jg, H, CW]),
            op=ALU.mult)
        for i in range(nt):
            t = t0 + i
            acc = accp.tile([P, FV], F32, tag="agg")
            nmm = NCHUNK * Jg
            mi = 0
            for k in range(NCHUNK):
                for j in range(Jg):
                    sl = ((k * nt + i) * Jg + j) * FV
                    nc.tensor.matmul(acc[:], lhsT=ident[:], rhs=rhs2[:, sl:sl + FV],
                                     start=(mi == 0), stop=(mi == nmm - 1))
                    mi += 1
            _finalize_tile(nc, fin, out_res, acc, zc, selfv, 0, vrep, vmul, t, CW, H)
        if progress is not None:
            progress(t0 + nt)


def _make_inputs(cfg, percore, waug1, waug2, inputs):
    wf = np.asarray(inputs["Wf"], np.float32).astype(NPBF16)
    b1r = np.broadcast_to(np.asarray(inputs["b1"], np.float32), (P, cfg.CW1)).copy()
    b2r = np.broadcast_to(np.asarray(inputs["b2"], np.float32), (P, cfg.C2)).copy()
    bfr = np.broadcast_to(np.asarray(inputs["bf"], np.float32), (P, cfg.NCOUT)).copy()
    waug1_bf = waug1.astype(NPBF16)
    waug2_bf = waug2.astype(NPBF16)
    in_maps = []
    for c in range(cfg.ncores):
        pc = percore[c]
        in_maps.append({
            "xexpT": pc["xexpT"], "selfv1": pc["selfv1"], "idxg": pc["idxg"],
            "waug1": waug1_bf, "waug2": waug2_bf, "wf": wf,
            "npad2": pc["npad2"], "ad1": pc["ad1"], "zc1": pc["zc1"],
            "b1r": b1r, "b2r": b2r, "bfr": bfr,
        })
    return in_maps


def _assemble(cfg, percore, results):
    out = np.zeros((cfg.NPAD, cfg.NCOUT), np.float32)
    for c in range(cfg.ncores):
        out[percore[c]["own"]] = results[c]["out"]
    return out[:cfg.N]


def kernel(**inputs) -> np.ndarray:
    cfg = Cfg(N=100000, E=800000, ncores=8)
    percore, waug1, waug2, meta = preprocess(
        cfg,
        np.asarray(inputs["x"], np.float32),
        np.asarray(inputs["edge_index"]),
        np.asarray(inputs["W1"], np.float32),
        np.asarray(inputs["a_src1"], np.float32),
        np.asarray(inputs["a_dst1"], np.float32),
        np.asarray(inputs["W2"], np.float32),
        np.asarray(inputs["a_src2"], np.float32),
        np.asarray(inputs["a_dst2"], np.float32),
    )
    nc = build_nc(cfg, meta)
    in_maps = _make_inputs(cfg, percore, waug1, waug2, inputs)
    res = run_bass_kernel_spmd(nc, in_maps, core_ids=list(range(cfg.ncores)))
    return _assemble(cfg, percore, res.results)


if __name__ == "__main__":
    import reference as R
    inputs = R.setup_inputs()
    out = kernel(**{k: np.asarray(v) for k, v in inputs.items()})
    print("out", out.shape, out.dtype)
